# revision 39
# baseline (speedup 1.0000x reference)
"""Trainium2 Bass kernel for nn_MixedAttnHeadEmbed (mixed-head-config attention).

Math (per batch b):
  Two attention configs share q_m/k_m/v_m [B,T,2048]:
    A: h=8  heads, d_max=256, mixing e in {1024,2048} -> d in {128,256}, weights w0,w1
    B: h=16 heads, d_max=128, mixing e in {1024,2048} -> d in {64,128},  weights w2,w3
  Each config: per-head q/k slices are RoPE'd, weight-summed (padded to d_max),
  GQA (8 kv heads), causal softmax attention; outputs of both configs sum.

Sharding: 8 cores = 4 batches x 2 shards. Shard s owns A-heads [4s,4s+4) and
B-heads [8s,8s+8) -> both write output columns [1024s, 1024s+1024) which are
summed on device; per-core output is the transposed block outT [1024, T] (fp16).

Device design notes (cost-model driven):
  * Everything on the elementwise path is fp16: DVE tensor_tensor gets the
    2x_1p fast mode, tensor_copy/tensor_scalar get 4x, DMA bytes halve, and
    fp16 matmuls stream at 1 cycle/row at ANY output width (fp32r pays 4x
    under 256).  Accumulation stays fp32 in PSUM; softmax input is fp32.
  * rotate_half operands arrive pre-permuted from HBM (sigma64/sigma32 row
    permutations are folded into extra DMA loads) so RoPE needs no on-chip
    shuffles or cross-partition copies: each mix is pure mult/add passes.
  * Scores are computed transposed (sT[k,q]) so softmax'd weights feed the
    y^T matmul directly; softmax is max-free (scores provably < 2), the
    denominator comes from an all-ones stationary matmul, and the causal
    diagonal is enforced by zeroing exp() outputs with affine_select (Pool,
    SBUF) instead of adding -1e9 to PSUM scores (DVE).
  * PSUM budget (8 banks): sT [P,1024]f32 double-buffered (4) + y (2) +
    den (2).  Config-A heads keep exp() outputs pt_c in SBUF and run the
    second output-d-chunk as a pure-matmul second pass over them.
  * ACT drains y PSUM tiles to SBUF fp16 copies so the single y region is
    released to the next accumulation chain at copy speed; DVE only does
    reciprocal + cheap fp16 normalize multiplies.
  * A ns-calibrated static balancer spreads mix passes across DVE/Pool
    (and ACT for copies) around the reserved exp/mask/normalize work.
"""

import math
from contextlib import ExitStack
from dataclasses import dataclass

import numpy as np

import concourse.bass as bass
import concourse.mybir as mybir
import concourse.tile as tile
from concourse import bacc

F32 = mybir.dt.float32
F16 = mybir.dt.float16
NEG = -1e9
P = 128


@dataclass(frozen=True)
class KCfg:
    T: int = 1024       # sequence length
    NA: int = 4         # config-A heads per core (d_max=256)
    NB: int = 8         # config-B heads per core (d_max=128); must be 2*NA

    @property
    def TK(self):
        return self.T // P

    @property
    def NKVB(self):
        return self.NB // 2

    @property
    def ROWS(self):
        return self.NA * 256  # == NB * 128 output rows per core


FULL = KCfg()


def _in_specs(cfg: KCfg):
    T = cfg.T
    na, nb = cfg.NA, cfg.NKVB
    return {
        # q/k transposed [cols, T]; *_s64/_s32 are rotate-half row permutations
        "qT1": (na * 128, T), "qT1s64": (na * 128, T), "qT1s32": (na * 128, T),
        "qT2": (na * 256, T), "qT2s64": (na * 256, T),
        "kTa1": (na * 128, T), "kTa1s64": (na * 128, T),
        "kTa2": (na * 256, T),
        "kTb1": (nb * 64, T), "kTb1s32": (nb * 64, T),
        # v pre-permuted per head: rows = head*P + p, cols = (chunk, d) flat
        "va1p": (na * P, (T // P) * 128),
        "va1pw3": (na * P, (T // P) * 128),
        "va2p": (na * P, (T // P) * 256),
        "vb1p": (nb * P, (T // P) * 64),
        # rope tables (weights & score scale folded, rot sign folded in sin)
        "tA1c": (128, T), "tA1s": (128, T),
        "tA2c": (256, T), "tA2s": (256, T),   # tA2s is half-SWAPPED (see host)
        "tB1c": (128, T), "tB1s": (128, T),
        "tB2c": (128, T), "tB2s": (128, T),
        "wvec": (P, 4),
    }


class _Pick:
    """Static ns-accurate load balancer across DVE / Pool(GPSIMD) / ACT.

    v1 cost model: engine time = free_size * cycle_t (DVE 1/0.96GHz, Pool &
    ACT 1/1.2GHz) with DVE fast modes: fp16 TT 2x, fp16 sbuf copy/TSP 4x.
    Init adders: DVE +60ns sbuf / +125ns psum, ACT +185ns, Pool ~+40ns."""

    def __init__(self, nc):
        self.nc = nc
        self.load = {"dve": 0.0, "pool": 0.0, "act": 0.0}

    # --- reservations for work that must sit on one engine ---
    def act_reserve(self, ns):
        self.load["act"] += ns
        return self.nc.scalar

    def pool_reserve(self, ns):
        self.load["pool"] += ns
        return self.nc.gpsimd

    def dve_reserve(self, ns):
        self.load["dve"] += ns
        return self.nc.vector

    # --- balanced ops ---
    def tt(self, out, a, b, op, free, fast=True, psum=False):
        d = free * 1.0417 * (0.5 if (fast and not psum) else 1.0) + (125 if psum else 60)
        if psum:
            self.load["dve"] += d
            self.nc.vector.tensor_tensor(out, a, b, op)
            return
        p = free * 0.8333 + 40
        if self.load["dve"] + d <= self.load["pool"] + p:
            self.load["dve"] += d
            self.nc.vector.tensor_tensor(out, a, b, op)
        else:
            self.load["pool"] += p
            self.nc.gpsimd.tensor_tensor(out, a, b, op)

    def copy(self, dst, src, free):
        costs = {"dve": free * 0.2604 + 60, "pool": free * 0.8333 + 40,
                 "act": free * 0.8333 + 217}
        eng = min(costs, key=lambda k: self.load[k] + costs[k])
        self.load[eng] += costs[eng]
        if eng == "act":
            self.nc.scalar.copy(dst, src)
        elif eng == "pool":
            self.nc.gpsimd.tensor_copy(dst, src)
        else:
            self.nc.vector.tensor_copy(dst, src)

    def tsp_mul(self, out, in0, scalar, free):
        self.load["dve"] += free * 0.2604 + 60
        self.nc.vector.tensor_scalar_mul(out, in0, scalar)

    def psum_drain(self, dst, src, free):
        """Copy a PSUM f32 tile to SBUF fp16: DVE or ACT (no fast modes)."""
        dcost = free * 1.0417 + 125
        acost = free * 0.8333 + 217
        if self.load["dve"] + dcost <= self.load["act"] + acost:
            self.load["dve"] += dcost
            self.nc.vector.tensor_copy(dst, src)
        else:
            self.load["act"] += acost
            self.nc.scalar.copy(dst, src)

    def stt(self, out, in0, scalar, in1, op0, op1, free):
        self.load["dve"] += free * 1.0417 + 60
        self.nc.vector.scalar_tensor_tensor(out, in0, scalar, in1, op0, op1)

    def tt_psum(self, out, a, b, op, free):
        """2-input op with a PSUM operand: DVE only."""
        self.load["dve"] += free * 1.0417 + 125
        self.nc.vector.tensor_tensor(out, a, b, op)


def build_program(cfg: KCfg = FULL):
    nc = bacc.Bacc("TRN2", target_bir_lowering=False)
    T, TK = cfg.T, cfg.TK
    mult, add = mybir.AluOpType.mult, mybir.AluOpType.add

    D = {}
    for name, shape in _in_specs(cfg).items():
        dt = F32 if name == "wvec" else F16
        D[name] = nc.declare_dram_parameter(name, list(shape), dt, isOutput=False)
    outT = nc.declare_dram_parameter("outT", [cfg.ROWS, T], F16, isOutput=True)
    RB = cfg.ROWS // P

    with ExitStack() as ctx:
        tc = ctx.enter_context(tile.TileContext(nc))
        const = ctx.enter_context(tc.tile_pool(name="const", bufs=1))
        rawp = ctx.enter_context(tc.tile_pool(name="raw", bufs=2))
        mixp = ctx.enter_context(tc.tile_pool(name="mix", bufs=2))
        scr = ctx.enter_context(tc.tile_pool(name="scr", bufs=2))
        ptp = ctx.enter_context(tc.tile_pool(name="pt", bufs=2))
        ycp = ctx.enter_context(tc.tile_pool(name="yc", bufs=2))
        recp = ctx.enter_context(tc.tile_pool(name="rec", bufs=2))
        accp = ctx.enter_context(tc.tile_pool(name="acc", bufs=1))
        keepp = ctx.enter_context(tc.tile_pool(name="keep", bufs=1))
        spsum = ctx.enter_context(tc.tile_pool(name="spsum", bufs=2, space="PSUM"))
        ypsum = ctx.enter_context(tc.tile_pool(name="ypsum", bufs=1, space="PSUM"))
        dpsum = ctx.enter_context(tc.tile_pool(name="dpsum", bufs=1, space="PSUM"))

        pick = _Pick(nc)
        keep = {}

        def get_shared(kind, src_ap, idx, dma_fn=None):
            """qT1/kTa1/kTa1s64 row-block idx is used by BOTH A-head idx and
            B-pair idx: load once, keep in SBUF for the whole kernel."""
            key = (kind, idx)
            if key not in keep:
                tl = keepp.tile([P, T], F16, tag=f"{kind}{idx}", name=f"{kind}{idx}")
                if dma_fn is None:
                    nc.sync.dma_start(out=tl, in_=src_ap)
                else:
                    dma_fn(tl, src_ap)
                keep[key] = tl
            return keep[key]

        # ---- constants ----
        ones_f = const.tile([P, P], F32, name="ones_f")
        nc.vector.memset(ones_f, 1.0)
        ones = const.tile([P, P], F16)
        nc.vector.tensor_copy(ones, ones_f)
        tabs = {}

        def load_tab(nm, qe=None):
            rows = _in_specs(cfg)[nm][0]
            tl = const.tile([P, rows // P, T], F16, name=nm, tag=nm)
            tabs[nm] = tl
            (qe or nc.sync).dma_start(out=tl, in_=D[nm].rearrange("(c p) t -> p c t", p=P))

        wv = const.tile([P, 4], F32)
        nc.sync.dma_start(out=wv, in_=D["wvec"][:, :])

        outacc = accp.tile([P, RB, T], F16)

        EXP = mybir.ActivationFunctionType.Exp

        def mix_A(out, x1, x1s, x2, c1, s1, c2, s2sw):
            """out [P,2,T] f16 = RoPE-mix of a config-A q or k head.
            x2 [P,2,T] (d=256), x1/x1s [P,T] (d=128, x1s sigma64-permuted).
            s2sw is the half-swapped signed d=256 sin table."""
            u2 = scr.tile([P, 2, T], F16, tag="u2")
            u1 = scr.tile([P, T], F16, tag="u1")
            u1b = scr.tile([P, T], F16, tag="u1b")
            pick.tt(out, x2, c2, mult, 2 * T)          # aligned cos products
            pick.tt(u2, x2, s2sw, mult, 2 * T)         # swapped sin products
            pick.tt(out[:, 0, :], out[:, 0, :], u2[:, 1, :], add, T)
            pick.tt(out[:, 1, :], out[:, 1, :], u2[:, 0, :], add, T)
            pick.tt(u1, x1, c1[:, 0, :], mult, T)
            pick.tt(u1b, x1s, s1[:, 0, :], mult, T)
            pick.tt(out[:, 0, :], out[:, 0, :], u1, add, T)
            pick.tt(out[:, 0, :], out[:, 0, :], u1b, add, T)

        BW = 512      # PSUM bank width in f32: matmul outs must stay in-bank
        NR = T // BW

        def pieces(c):
            q0 = P * c
            return [(r, max(BW * r, q0), BW * (r + 1))
                    for r in range(NR) if BW * (r + 1) > max(BW * r, q0)]

        def last_c(r):
            return min(TK, (BW // P) * (r + 1)) - 1

        def pieces_diag_last(c):
            """y/den matmul pieces with the causal-diagonal columns last,
            giving the Pool-side pt mask time to land off PE's critical path."""
            q0 = P * c
            out = []
            for (r, lo, hi) in pieces(c):
                if lo == q0:
                    if hi > q0 + P:
                        out.append((r, q0 + P, hi))
                else:
                    out.append((r, lo, hi))
            out.append((q0 // BW, q0, q0 + P))
            return out

        def flush(blk):
            nc.sync.dma_start(out=outT[P * blk:P * (blk + 1), :],
                              in_=outacc[:, blk, :])

        def attn(q_aps, k_aps, v_lhs, blks, store, do_flush):
            """q_aps/k_aps: per-d-chunk [P,T] f16 APs (d on partitions).
            v_lhs(c, vc): stationary [P, d] AP for k-chunk c, out-chunk vc.
            blks: output 128-row blocks (1 for B, 2 for A).  y and den live
            in per-bank PSUM tiles so each bank's chain (low closes at c=3)
            releases its readers early despite tile-granular dep tracking."""
            ndc = len(q_aps)
            denB = [dpsum.tile([P, BW], F32, tag=f"den{r}", name=f"den{r}")
                    for r in range(NR)]
            ytB = [ypsum.tile([P, BW], F32, tag=f"yt{r}", name=f"yt{r}")
                   for r in range(NR)]
            pts = []
            sTs = {}

            def emit_sT(c):
                q0 = P * c
                sT = spsum.tile([P, T], F32, tag="sT")
                for (r, lo, hi) in pieces(c):
                    for dc in range(ndc):
                        nc.tensor.matmul(
                            sT[:, lo:hi], k_aps[dc][:, q0:q0 + P], q_aps[dc][:, lo:hi],
                            start=(dc == 0), stop=(dc == ndc - 1))
                sTs[c] = sT

            # software pipeline: sT(c+1) is issued BEFORE y/den(c) so PE
            # computes next-chunk scores while ACT runs exp(c) instead of
            # stalling in-order on the exp -> y dependency.
            emit_sT(0)
            for c in range(TK):
                if c + 1 < TK:
                    emit_sT(c + 1)
                q0 = P * c
                n = T - q0
                sT = sTs.pop(c)
                pt = ptp.tile([P, n], F16, tag=f"pt{c}", name=f"pt{c}")
                pick.act_reserve(n * 0.8333 + 185).activation(pt, sT[:, q0:], EXP)
                # causal diagonal: zero exp() where q < k.  For c==0 the diag
                # must be masked in place (its matmul carries start=True and
                # must cover the full bank in one piece); for c>0 mask into a
                # separate tile and order the diag piece last so the nondiag
                # matmuls depend only on exp, not on the Pool-side mask.
                if c == 0:
                    pick.pool_reserve(150).affine_select(
                        out=pt[:, 0:P], in_=pt[:, 0:P],
                        compare_op=mybir.AluOpType.is_ge, fill=0.0,
                        base=0, pattern=[[1, P]], channel_multiplier=-1)
                    ptd = None
                    plist = pieces(c)
                else:
                    ptd = ptp.tile([P, P], F16, tag=f"ptd{c}", name=f"ptd{c}")
                    pick.pool_reserve(150).affine_select(
                        out=ptd, in_=pt[:, 0:P],
                        compare_op=mybir.AluOpType.is_ge, fill=0.0,
                        base=0, pattern=[[1, P]], channel_multiplier=-1)
                    plist = pieces_diag_last(c)
                for (r, lo, hi) in plist:
                    src_ = (ptd if (ptd is not None and lo == q0)
                            else pt[:, lo - q0:hi - q0])
                    nc.tensor.matmul(ytB[r][:, lo - BW * r:hi - BW * r],
                                     v_lhs(c, 0), src_,
                                     start=(c == 0), stop=(c == last_c(r)))
                    nc.tensor.matmul(denB[r][:, lo - BW * r:hi - BW * r],
                                     ones, src_,
                                     start=(c == 0), stop=(c == last_c(r)))
                pts.append((pt, ptd))
            rec = recp.tile([P, T], F16, tag="rec")
            with nc.allow_low_precision(reason="1/den fits fp16; den in [1, 8e3]"):
                for r in range(NR):
                    pick.dve_reserve(BW * 1.0417 + 125).reciprocal(
                        rec[:, BW * r:BW * (r + 1)], denB[r])
            if len(blks) == 2:
                yc0 = ycp.tile([P, T], F16, tag="yc")
                for r in range(NR):
                    pick.psum_drain(yc0[:, BW * r:BW * (r + 1)], ytB[r], BW)
                yt2B = [ypsum.tile([P, BW], F32, tag=f"yt{r}", name=f"yt{r}b")
                        for r in range(NR)]
                for c in range(TK):
                    q0 = P * c
                    plist = pieces(c) if c == 0 else pieces_diag_last(c)
                    for (r, lo, hi) in plist:
                        src_ = (pts[c][1] if (pts[c][1] is not None and lo == q0)
                                else pts[c][0][:, lo - q0:hi - q0])
                        nc.tensor.matmul(yt2B[r][:, lo - BW * r:hi - BW * r],
                                         v_lhs(c, 1), src_,
                                         start=(c == 0), stop=(c == last_c(r)))
                yc1 = ycp.tile([P, T], F16, tag="yc")
                for r in range(NR):
                    pick.psum_drain(yc1[:, BW * r:BW * (r + 1)], yt2B[r], BW)
                for bi, yc in ((0, yc0), (1, yc1)):
                    if store:
                        pick.tt(outacc[:, blks[bi], :], yc, rec, mult, T)
                    else:
                        tmp = scr.tile([P, T], F16, tag="btmp")
                        pick.tt(tmp, yc, rec, mult, T)
                        pick.tt(outacc[:, blks[bi], :], outacc[:, blks[bi], :],
                                tmp, add, T)
                    if do_flush:
                        flush(blks[bi])
            else:
                if store:
                    for r in range(NR):
                        sl = slice(BW * r, BW * (r + 1))
                        pick.tt_psum(outacc[:, blks[0], sl], ytB[r], rec[:, sl], mult, BW)
                else:
                    tmp = scr.tile([P, T], F16, tag="btmp")
                    for r in range(NR):
                        sl = slice(BW * r, BW * (r + 1))
                        pick.tt_psum(tmp[:, sl], ytB[r], rec[:, sl], mult, BW)
                    for r in range(NR):
                        sl = slice(BW * r, BW * (r + 1))
                        pick.tt(outacc[:, blks[0], sl], outacc[:, blks[0], sl],
                                tmp[:, sl], add, BW)
                        if do_flush:  # flush each half as soon as it's ready
                            b = blks[0]
                            nc.sync.dma_start(out=outT[P * b:P * (b + 1), sl],
                                              in_=outacc[:, b, sl])
                    return
                if do_flush:
                    flush(blks[0])

        # ---- per-head builders --------------------------------------
        def do_A(i, store, do_flush, warm=False):
            qeng = nc.scalar if warm else nc.sync
            r1 = slice(P * i, P * (i + 1))
            r2 = slice(256 * i, 256 * (i + 1))
            q1 = get_shared("sq1", D["qT1"][r1, :], i,
                            lambda o, a: qeng.dma_start(out=o, in_=a))
            q1s = rawp.tile([P, T], F16, tag="q1s")
            qeng.dma_start(out=q1s, in_=D["qT1s64"][r1, :])
            q2 = rawp.tile([P, 2, T], F16, tag="q2")
            qeng.dma_start(out=q2, in_=D["qT2"][r2, :].rearrange("(c p) t -> p c t", p=P))
            qmix = mixp.tile([P, 2, T], F16, tag="qmix")
            mix_A(qmix, q1, q1s, q2, tabs["tA1c"], tabs["tA1s"], tabs["tA2c"], tabs["tA2s"])

            k1 = get_shared("sk1", D["kTa1"][r1, :], i)
            k1s = get_shared("sk1s", D["kTa1s64"][r1, :], i)
            k2 = rawp.tile([P, 2, T], F16, tag="k2")
            nc.sync.dma_start(out=k2, in_=D["kTa2"][r2, :].rearrange("(c p) t -> p c t", p=P))
            kmix = mixp.tile([P, 2, T], F16, tag="kmix")
            mix_A(kmix, k1, k1s, k2, tabs["tA1c"], tabs["tA1s"], tabs["tA2c"], tabs["tA2s"])

            v1 = rawp.tile([P, TK, P], F16, tag="v1")
            nc.sync.dma_start(out=v1, in_=D["va1p"][r1, :].rearrange("p (c d) -> p c d", d=P))
            vmix = rawp.tile([P, TK, 2 * P], F16, tag="v2")
            nc.sync.dma_start(out=vmix, in_=D["va2p"][r1, :].rearrange("p (c d) -> p c d", d=2 * P))
            # va2p arrives host-prescaled by w1; fold w0*v1 in-place
            pick.stt(vmix[:, :, 0:P], v1, wv[:, 0:1], vmix[:, :, 0:P],
                     mult, add, T)

            attn([qmix[:, 0, :], qmix[:, 1, :]],
                 [kmix[:, 0, :], kmix[:, 1, :]],
                 lambda c, vc: vmix[:, c, P * vc:P * (vc + 1)],
                 (2 * i, 2 * i + 1), store=store, do_flush=do_flush)

        tp_hold = {}

        def do_B(j, store, do_flush, warm=False):
            rj = slice(P * j, P * (j + 1))
            # warm mode: round-robin the loads over four engine DMA queues so
            # the first head's operands stream in parallel instead of
            # serializing on SP (the model charges transfers per-queue).
            wq = {"k2": nc.sync, "k2s": nc.gpsimd, "k1p": nc.scalar,
                  "k1ps": nc.sync, "q1p": nc.sync, "q1ps": nc.sync,
                  "q2p": nc.scalar, "q2ps": nc.sync, "vmix": nc.sync,
                  "v1": nc.sync}
            wcost = {id(nc.scalar): "act", id(nc.gpsimd): "pool"}

            def dma(out_, in_, key=None):
                eng = wq.get(key, nc.sync) if warm else nc.sync
                eng.dma_start(out=out_, in_=in_)
                if warm and id(eng) in wcost:  # DMA occupies that queue
                    pick.load[wcost[id(eng)]] += out_.free_size() * 2 * 0.3855

            k2 = get_shared("sk1", D["kTa1"][rj, :], j,
                            lambda o, a: dma(o, a, "k2"))
            k2s = get_shared("sk1s", D["kTa1s64"][rj, :], j,
                             lambda o, a: dma(o, a, "k2s"))
            u0 = j // 2
            if j % 2 == 0:
                k1p = rawp.tile([P, T], F16, tag="bk1p")
                dma(k1p, D["kTb1"][P * u0:P * (u0 + 1), :], "k1p")
                k1ps = rawp.tile([P, T], F16, tag="bk1ps")
                dma(k1ps, D["kTb1s32"][P * u0:P * (u0 + 1), :], "k1ps")
            # q pair for heads (2j, 2j+1)
            q1p = get_shared("sq1", D["qT1"][rj, :], j,
                             lambda o, a: dma(o, a, "q1p"))
            q1ps = rawp.tile([P, T], F16, tag="k1s")
            dma(q1ps, D["qT1s32"][rj, :], "q1ps")
            q2p = rawp.tile([P, 2, T], F16, tag="q2")
            dma(q2p, D["qT2"][256 * j:256 * (j + 1), :]
                .rearrange("(c p) t -> p c t", p=P), "q2p")
            q2ps = rawp.tile([P, 2, T], F16, tag="k2")
            dma(q2ps, D["qT2s64"][256 * j:256 * (j + 1), :]
                .rearrange("(c p) t -> p c t", p=P), "q2ps")
            vmix = rawp.tile([P, TK, P], F16, tag="v1")
            dma(vmix, D["va1pw3"][rj, :].rearrange("p (c d) -> p c d", d=P), "vmix")
            v1 = rawp.tile([P, TK, 64], F16, tag="bv1")
            dma(v1, D["vb1p"][rj, :].rearrange("p (c d) -> p c d", d=64), "v1")

            kmix = mixp.tile([P, T], F16, tag="bkmix")
            u = scr.tile([P, T], F16, tag="u1")
            pick.tt(kmix, k2, tabs["tB2c"][:, 0, :], mult, T)
            pick.tt(u, k2s, tabs["tB2s"][:, 0, :], mult, T)
            pick.tt(kmix, kmix, u, add, T)
            if j % 2 == 0:
                tp = scr.tile([P, T], F16, tag="btp")
                tpb = scr.tile([P, T], F16, tag="btpb")
                pick.tt(tp, k1p, tabs["tB1c"][:, 0, :], mult, T)
                pick.tt(tpb, k1ps, tabs["tB1s"][:, 0, :], mult, T)
                pick.tt(tp, tp, tpb, add, T)
                tp_hold[0] = tp
                pick.tt(kmix[0:64, :], kmix[0:64, :], tp[0:64, :], add, T)
            else:
                tc2 = scr.tile([P, T], F16, tag="btc")
                pick.copy(tc2[0:64, :], tp_hold[0][64:128, :], T)
                pick.tt(kmix[0:64, :], kmix[0:64, :], tc2[0:64, :], add, T)

            qp = mixp.tile([P, 2, T], F16, tag="bqp")
            uq = scr.tile([P, 2, T], F16, tag="u2")
            t1 = scr.tile([P, T], F16, tag="u1")
            t1b = scr.tile([P, T], F16, tag="u1b")
            pick.tt(t1, q1p, tabs["tB1c"][:, 0, :], mult, T)
            pick.tt(t1b, q1ps, tabs["tB1s"][:, 0, :], mult, T)
            pick.tt(t1, t1, t1b, add, T)
            for hh in range(2):  # head 0's column completes first
                pick.tt(qp[:, hh, :], q2p[:, hh, :], tabs["tB2c"][:, 0, :],
                        mult, T)
                pick.tt(uq[:, hh, :], q2ps[:, hh, :], tabs["tB2s"][:, 0, :],
                        mult, T)
                pick.tt(qp[:, hh, :], qp[:, hh, :], uq[:, hh, :], add, T)
                if hh == 0:
                    pick.tt(qp[0:64, 0, :], qp[0:64, 0, :], t1[0:64, :], add, T)
                else:
                    tcq = scr.tile([P, T], F16, tag="btc")
                    pick.copy(tcq[0:64, :], t1[64:128, :], T)
                    pick.tt(qp[0:64, 1, :], qp[0:64, 1, :], tcq[0:64, :], add, T)

            # va1pw3 is host-prescaled by w3, vb1p by w2: one in-place add
            pick.tt(vmix[:, :, 0:64], vmix[:, :, 0:64], v1, add, T // 2)

            for hh in range(2):
                attn([qp[:, hh, :]], [kmix],
                     lambda c, vc: vmix[:, c, :],
                     (2 * j + hh,), store=store, do_flush=do_flush)

        # ---- schedule: B pair 0 first (short mix chain covers PE warmup),
        # then A heads (A0 adds onto B0's blocks), then remaining B pairs.
        load_tab("tB2c", nc.gpsimd)
        load_tab("tB2s", nc.scalar)
        load_tab("tB1c", nc.gpsimd)
        load_tab("tB1s", nc.gpsimd)
        pick.load["pool"] += 3 * 790
        pick.load["act"] += 790
        do_B(0, store=True, do_flush=False, warm=True)
        for nm in ("tA1c", "tA1s", "tA2c", "tA2s"):
            load_tab(nm)
        do_A(0, store=False, do_flush=True, warm=True)
        for i in range(1, cfg.NA):
            do_B(i, store=True, do_flush=False)
            do_A(i, store=False, do_flush=True)

    nc.compile()
    return nc


# ---------------------------------------------------------------------------
# Host side
# ---------------------------------------------------------------------------

def _rope_tab(pos, d, f):
    """Transposed rope tables [d, T]: (f*cos, f*sin with rot sign folded)."""
    inv = 1.0 / (10000.0 ** (np.arange(0, d, 2, dtype=np.float32) / d))
    ang = inv[:, None] * pos[None, :].astype(np.float32)      # [d/2, T]
    ang = np.concatenate([ang, ang], 0)                        # [d, T]
    c = (f * np.cos(ang)).astype(np.float32)
    s = (f * np.sin(ang)).astype(np.float32)
    s[: d // 2] *= -1.0
    return c, s


def _sig(a, half):
    """Row permutation: swap halves of size `half` in each 2*half group."""
    out = a.reshape(-1, 2, half, a.shape[-1])[:, ::-1]
    return out.reshape(a.shape)


def _vperm(vslc, dh):
    """[T, nh*dh] -> [nh, P, T//P, dh] contiguous per-partition rows."""
    T = vslc.shape[0]
    nh = vslc.shape[1] // dh
    # [c, p, head, d] -> [head, p, c, d]
    return vslc.reshape(T // P, P, nh, dh).transpose(2, 1, 0, 3)


def make_core_inputs(q, k, v, pos, weights, s, cfg: KCfg = FULL):
    """q,k,v: [T, 2048] fp32 for one batch; returns the per-core input dict."""
    f16 = lambda a: np.ascontiguousarray(a, dtype=np.float16)
    qT1 = q[:, 512 * s:512 * s + 512].T
    qT2 = q[:, 1024 * s:1024 * s + 1024].T
    kTa1 = k[:, 512 * s:512 * s + 512].T
    kTa2 = k[:, 1024 * s:1024 * s + 1024].T
    kTb1 = k[:, 256 * s:256 * s + 256].T
    arrs = {
        "qT1": f16(qT1), "qT1s64": f16(_sig(qT1, 64)), "qT1s32": f16(_sig(qT1, 32)),
        "qT2": f16(qT2), "qT2s64": f16(_sig(qT2, 64)),
        "kTa1": f16(kTa1), "kTa1s64": f16(_sig(kTa1, 64)),
        "kTa2": f16(kTa2),
        "kTb1": f16(kTb1), "kTb1s32": f16(_sig(kTb1, 32)),
        "va1p": f16(_vperm(v[:, 512 * s:512 * s + 512], 128).reshape(4 * P, -1)),
        "va1pw3": f16(float(weights[3])
                      * _vperm(v[:, 512 * s:512 * s + 512], 128).reshape(4 * P, -1)),
        "va2p": f16(float(weights[1])
                    * _vperm(v[:, 1024 * s:1024 * s + 1024], 256).reshape(4 * P, -1)),
        "vb1p": f16(float(weights[2])
                    * _vperm(v[:, 256 * s:256 * s + 256], 64).reshape(4 * P, -1)),
    }
    fA = math.sqrt(1.0 / 16.0)
    fB = math.sqrt(1.0 / math.sqrt(128.0))
    c1, s1 = _rope_tab(pos, 128, fA * float(weights[0]))
    c2, s2 = _rope_tab(pos, 256, fA * float(weights[1]))
    cb1h, sb1h = _rope_tab(pos, 64, fB * float(weights[2]))
    cb2, sb2 = _rope_tab(pos, 128, fB * float(weights[3]))
    arrs.update({
        "tA1c": f16(c1), "tA1s": f16(s1),
        # tA2s half-swapped: row block 0 holds the sin factors for x2[:,0,:]
        # (which contribute to out dim-chunk 1), block 1 those for x2[:,1,:].
        "tA2c": f16(c2), "tA2s": f16(np.vstack([s2[128:], s2[:128]])),
        "tB1c": f16(np.vstack([cb1h, cb1h])), "tB1s": f16(np.vstack([sb1h, sb1h])),
        "tB2c": f16(cb2), "tB2s": f16(sb2),
        "wvec": np.tile(np.asarray(weights, np.float32)[None, :], (P, 1)),
    })
    return arrs


_PROGRAM_CACHE = {}
TRACE = False
LAST_RESULT = None


def kernel(q_m, k_m, v_m, weights, attention_mask, position_ids):
    global LAST_RESULT
    from concourse.bass_utils import run_bass_kernel_spmd

    cfg = FULL
    q_m = np.asarray(q_m, np.float32)
    k_m = np.asarray(k_m, np.float32)
    v_m = np.asarray(v_m, np.float32)
    weights = np.asarray(weights, np.float32)
    attention_mask = np.asarray(attention_mask, np.float32)
    position_ids = np.asarray(position_ids)
    B, T, H = q_m.shape

    # the device program hardcodes the causal structure; verify it holds
    causal = np.where(np.tril(np.ones((T, T), bool)), 0.0, NEG).astype(np.float32)
    for b in range(B):
        assert np.array_equal(attention_mask[b, 0], causal), "non-causal mask"

    if "nc" not in _PROGRAM_CACHE:
        _PROGRAM_CACHE["nc"] = build_program(cfg)
    nc = _PROGRAM_CACHE["nc"]

    in_maps = []
    for b in range(B):
        for s in range(2):
            in_maps.append(make_core_inputs(
                q_m[b], k_m[b], v_m[b], position_ids[b], weights, s, cfg))
    res = run_bass_kernel_spmd(nc, in_maps, list(range(8)), trace=TRACE)
    LAST_RESULT = res
    out = np.zeros((B, T, H), np.float32)
    for b in range(B):
        for s in range(2):
            out[b, :, 1024 * s:1024 * s + 1024] = \
                res.results[2 * b + s]["outT"].astype(np.float32).T
    return out


# revision 41
# speedup vs baseline: 1.0085x; 1.0085x over previous
"""Trainium2 Bass kernel for nn_MixedAttnHeadEmbed (mixed-head-config attention).

Math (per batch b):
  Two attention configs share q_m/k_m/v_m [B,T,2048]:
    A: h=8  heads, d_max=256, mixing e in {1024,2048} -> d in {128,256}, weights w0,w1
    B: h=16 heads, d_max=128, mixing e in {1024,2048} -> d in {64,128},  weights w2,w3
  Each config: per-head q/k slices are RoPE'd, weight-summed (padded to d_max),
  GQA (8 kv heads), causal softmax attention; outputs of both configs sum.

Sharding: 8 cores = 4 batches x 2 shards. Shard s owns A-heads [4s,4s+4) and
B-heads [8s,8s+8) -> both write output columns [1024s, 1024s+1024) which are
summed on device; per-core output is the transposed block outT [1024, T] (fp16).

Device design notes (cost-model driven):
  * Everything on the elementwise path is fp16: DVE tensor_tensor gets the
    2x_1p fast mode, tensor_copy/tensor_scalar get 4x, DMA bytes halve, and
    fp16 matmuls stream at 1 cycle/row at ANY output width (fp32r pays 4x
    under 256).  Accumulation stays fp32 in PSUM; softmax input is fp32.
  * rotate_half operands arrive pre-permuted from HBM (sigma64/sigma32 row
    permutations are folded into extra DMA loads) so RoPE needs no on-chip
    shuffles or cross-partition copies: each mix is pure mult/add passes.
  * Scores are computed transposed (sT[k,q]) so softmax'd weights feed the
    y^T matmul directly; softmax is max-free (scores provably < 2), the
    denominator comes from an all-ones stationary matmul, and the causal
    diagonal is enforced by zeroing exp() outputs with affine_select (Pool,
    SBUF) instead of adding -1e9 to PSUM scores (DVE).
  * PSUM budget (8 banks): sT [P,1024]f32 double-buffered (4) + y (2) +
    den (2).  Config-A heads keep exp() outputs pt_c in SBUF and run the
    second output-d-chunk as a pure-matmul second pass over them.
  * ACT drains y PSUM tiles to SBUF fp16 copies so the single y region is
    released to the next accumulation chain at copy speed; DVE only does
    reciprocal + cheap fp16 normalize multiplies.
  * A ns-calibrated static balancer spreads mix passes across DVE/Pool
    (and ACT for copies) around the reserved exp/mask/normalize work.
"""

import math
from contextlib import ExitStack
from dataclasses import dataclass

import numpy as np

import concourse.bass as bass
import concourse.mybir as mybir
import concourse.tile as tile
from concourse import bacc

F32 = mybir.dt.float32
F16 = mybir.dt.float16
F8 = mybir.dt.float8e4
NEG = -1e9
P = 128


@dataclass(frozen=True)
class KCfg:
    T: int = 1024       # sequence length
    NA: int = 4         # config-A heads per core (d_max=256)
    NB: int = 8         # config-B heads per core (d_max=128); must be 2*NA

    @property
    def TK(self):
        return self.T // P

    @property
    def NKVB(self):
        return self.NB // 2

    @property
    def ROWS(self):
        return self.NA * 256  # == NB * 128 output rows per core


FULL = KCfg()


def _in_specs(cfg: KCfg):
    T = cfg.T
    na, nb = cfg.NA, cfg.NKVB
    return {
        # q/k transposed [cols, T]; *_s64/_s32 are rotate-half row permutations
        "qT1": (na * 128, T), "qT1s64": (na * 128, T), "qT1s32": (na * 128, T),
        "qT2": (na * 256, T), "qT2s64": (na * 256, T),
        "kTa1": (na * 128, T), "kTa1s64": (na * 128, T),
        "kTa2": (na * 256, T),
        "kTb1": (nb * 64, T), "kTb1s32": (nb * 64, T),
        # v pre-permuted per head: rows = head*P + p, cols = (chunk, d) flat
        "va1p": (na * P, (T // P) * 128),
        "va1pw3": (na * P, (T // P) * 128),
        "va2p": (na * P, (T // P) * 256),
        "vb1p": (nb * P, (T // P) * 64),
        # rope tables (weights & score scale folded, rot sign folded in sin)
        "tA1c": (128, T), "tA1s": (128, T),
        "tA2c": (256, T), "tA2s": (256, T),   # tA2s is half-SWAPPED (see host)
        "tB1c": (128, T), "tB1s": (128, T),
        "tB2c": (128, T), "tB2s": (128, T),
        "wvec": (P, 4),
    }


class _Pick:
    """Static ns-accurate load balancer across DVE / Pool(GPSIMD) / ACT.

    v1 cost model: engine time = free_size * cycle_t (DVE 1/0.96GHz, Pool &
    ACT 1/1.2GHz) with DVE fast modes: fp16 TT 2x, fp16 sbuf copy/TSP 4x.
    Init adders: DVE +60ns sbuf / +125ns psum, ACT +185ns, Pool ~+40ns."""

    def __init__(self, nc):
        self.nc = nc
        self.load = {"dve": 0.0, "pool": 0.0, "act": 0.0}

    # --- reservations for work that must sit on one engine ---
    def act_reserve(self, ns):
        self.load["act"] += ns
        return self.nc.scalar

    def pool_reserve(self, ns):
        self.load["pool"] += ns
        return self.nc.gpsimd

    def dve_reserve(self, ns):
        self.load["dve"] += ns
        return self.nc.vector

    # --- balanced ops ---
    def tt(self, out, a, b, op, free, fast=True, psum=False):
        d = free * 1.0417 * (0.5 if (fast and not psum) else 1.0) + (125 if psum else 60)
        if psum:
            self.load["dve"] += d
            self.nc.vector.tensor_tensor(out, a, b, op)
            return
        p = free * 0.8333 + 40
        if self.load["dve"] + d <= self.load["pool"] + p:
            self.load["dve"] += d
            self.nc.vector.tensor_tensor(out, a, b, op)
        else:
            self.load["pool"] += p
            self.nc.gpsimd.tensor_tensor(out, a, b, op)

    def copy(self, dst, src, free):
        costs = {"dve": free * 0.2604 + 60, "pool": free * 0.8333 + 40,
                 "act": free * 0.8333 + 217}
        eng = min(costs, key=lambda k: self.load[k] + costs[k])
        self.load[eng] += costs[eng]
        if eng == "act":
            self.nc.scalar.copy(dst, src)
        elif eng == "pool":
            self.nc.gpsimd.tensor_copy(dst, src)
        else:
            self.nc.vector.tensor_copy(dst, src)

    def tsp_mul(self, out, in0, scalar, free):
        self.load["dve"] += free * 0.2604 + 60
        self.nc.vector.tensor_scalar_mul(out, in0, scalar)

    def psum_drain(self, dst, src, free):
        """Copy a PSUM f32 tile to SBUF fp16: DVE or ACT (no fast modes)."""
        dcost = free * 1.0417 + 125
        acost = free * 0.8333 + 217
        if self.load["dve"] + dcost <= self.load["act"] + acost:
            self.load["dve"] += dcost
            self.nc.vector.tensor_copy(dst, src)
        else:
            self.load["act"] += acost
            self.nc.scalar.copy(dst, src)

    def stt(self, out, in0, scalar, in1, op0, op1, free):
        self.load["dve"] += free * 1.0417 + 60
        self.nc.vector.scalar_tensor_tensor(out, in0, scalar, in1, op0, op1)

    def tt_psum(self, out, a, b, op, free):
        """2-input op with a PSUM operand: DVE only."""
        self.load["dve"] += free * 1.0417 + 125
        self.nc.vector.tensor_tensor(out, a, b, op)


def build_program(cfg: KCfg = FULL):
    nc = bacc.Bacc("TRN2", target_bir_lowering=False)
    T, TK = cfg.T, cfg.TK
    mult, add = mybir.AluOpType.mult, mybir.AluOpType.add

    D = {}
    for name, shape in _in_specs(cfg).items():
        dt = F32 if name == "wvec" else F16
        D[name] = nc.declare_dram_parameter(name, list(shape), dt, isOutput=False)
    outT = nc.declare_dram_parameter("outT", [cfg.ROWS, T], F16, isOutput=True)
    RB = cfg.ROWS // P

    with ExitStack() as ctx:
        tc = ctx.enter_context(tile.TileContext(nc))
        const = ctx.enter_context(tc.tile_pool(name="const", bufs=1))
        rawp = ctx.enter_context(tc.tile_pool(name="raw", bufs=2))
        mixp = ctx.enter_context(tc.tile_pool(name="mix", bufs=2))
        scr = ctx.enter_context(tc.tile_pool(name="scr", bufs=2))
        ptp = ctx.enter_context(tc.tile_pool(name="pt", bufs=2))
        ycp = ctx.enter_context(tc.tile_pool(name="yc", bufs=2))
        recp = ctx.enter_context(tc.tile_pool(name="rec", bufs=2))
        accp = ctx.enter_context(tc.tile_pool(name="acc", bufs=1))
        keepp = ctx.enter_context(tc.tile_pool(name="keep", bufs=1))
        spsum = ctx.enter_context(tc.tile_pool(name="spsum", bufs=2, space="PSUM"))
        ypsum = ctx.enter_context(tc.tile_pool(name="ypsum", bufs=1, space="PSUM"))
        dpsum = ctx.enter_context(tc.tile_pool(name="dpsum", bufs=1, space="PSUM"))

        pick = _Pick(nc)
        keep = {}

        def get_shared(kind, src_ap, idx, dma_fn=None):
            """qT1/kTa1/kTa1s64 row-block idx is used by BOTH A-head idx and
            B-pair idx: load once, keep in SBUF for the whole kernel."""
            key = (kind, idx)
            if key not in keep:
                tl = keepp.tile([P, T], F16, tag=f"{kind}{idx}", name=f"{kind}{idx}")
                if dma_fn is None:
                    nc.sync.dma_start(out=tl, in_=src_ap)
                else:
                    dma_fn(tl, src_ap)
                keep[key] = tl
            return keep[key]

        # ---- constants ----
        ones_f = const.tile([P, P], F32, name="ones_f")
        nc.vector.memset(ones_f, 1.0)
        ones = const.tile([P, P], F16)
        nc.vector.tensor_copy(ones, ones_f)
        tabs = {}

        def load_tab(nm, qe=None):
            rows = _in_specs(cfg)[nm][0]
            tl = const.tile([P, rows // P, T], F16, name=nm, tag=nm)
            tabs[nm] = tl
            (qe or nc.sync).dma_start(out=tl, in_=D[nm].rearrange("(c p) t -> p c t", p=P))

        wv = const.tile([P, 4], F32)
        nc.sync.dma_start(out=wv, in_=D["wvec"][:, :])

        outacc = accp.tile([P, RB, T], F16)

        EXP = mybir.ActivationFunctionType.Exp

        def mix_A(out, x1, x1s, x2, c1, s1, c2, s2sw):
            """out [P,2,T] f16 = RoPE-mix of a config-A q or k head.
            x2 [P,2,T] (d=256), x1/x1s [P,T] (d=128, x1s sigma64-permuted).
            s2sw is the half-swapped signed d=256 sin table."""
            u2 = scr.tile([P, 2, T], F16, tag="u2")
            u1 = scr.tile([P, T], F16, tag="u1")
            u1b = scr.tile([P, T], F16, tag="u1b")
            pick.tt(out, x2, c2, mult, 2 * T)          # aligned cos products
            pick.tt(u2, x2, s2sw, mult, 2 * T)         # swapped sin products
            pick.tt(out[:, 0, :], out[:, 0, :], u2[:, 1, :], add, T)
            pick.tt(out[:, 1, :], out[:, 1, :], u2[:, 0, :], add, T)
            pick.tt(u1, x1, c1[:, 0, :], mult, T)
            pick.tt(u1b, x1s, s1[:, 0, :], mult, T)
            pick.tt(out[:, 0, :], out[:, 0, :], u1, add, T)
            pick.tt(out[:, 0, :], out[:, 0, :], u1b, add, T)

        BW = 512      # PSUM bank width in f32: matmul outs must stay in-bank
        NR = T // BW

        def pieces(c):
            q0 = P * c
            return [(r, max(BW * r, q0), BW * (r + 1))
                    for r in range(NR) if BW * (r + 1) > max(BW * r, q0)]

        def last_c(r):
            return min(TK, (BW // P) * (r + 1)) - 1

        def pieces_diag_last(c):
            """y/den matmul pieces with the causal-diagonal columns last,
            giving the Pool-side pt mask time to land off PE's critical path."""
            q0 = P * c
            out = []
            for (r, lo, hi) in pieces(c):
                if lo == q0:
                    if hi > q0 + P:
                        out.append((r, q0 + P, hi))
                else:
                    out.append((r, lo, hi))
            out.append((q0 // BW, q0, q0 + P))
            return out

        def flush(blk):
            nc.sync.dma_start(out=outT[P * blk:P * (blk + 1), :],
                              in_=outacc[:, blk, :])

        def attn(q_aps, k_aps, v_lhs, blks, store, do_flush):
            """q_aps/k_aps: per-d-chunk [P,T] f16 APs (d on partitions).
            v_lhs(c, vc): stationary [P, d] AP for k-chunk c, out-chunk vc.
            blks: output 128-row blocks (1 for B, 2 for A).  y and den live
            in per-bank PSUM tiles so each bank's chain (low closes at c=3)
            releases its readers early despite tile-granular dep tracking."""
            ndc = len(q_aps)
            denB = [dpsum.tile([P, BW], F32, tag=f"den{r}", name=f"den{r}")
                    for r in range(NR)]
            ytB = [ypsum.tile([P, BW], F32, tag=f"yt{r}", name=f"yt{r}")
                   for r in range(NR)]
            pts = []
            sTs = {}

            def emit_sT(c):
                q0 = P * c
                sT = spsum.tile([P, T], F32, tag="sT")
                for (r, lo, hi) in pieces(c):
                    for dc in range(ndc):
                        nc.tensor.matmul(
                            sT[:, lo:hi], k_aps[dc][:, q0:q0 + P], q_aps[dc][:, lo:hi],
                            start=(dc == 0), stop=(dc == ndc - 1))
                sTs[c] = sT

            # software pipeline: sT(c+1) is issued BEFORE y/den(c) so PE
            # computes next-chunk scores while ACT runs exp(c) instead of
            # stalling in-order on the exp -> y dependency.
            emit_sT(0)
            for c in range(TK):
                if c + 1 < TK:
                    emit_sT(c + 1)
                q0 = P * c
                n = T - q0
                sT = sTs.pop(c)
                pt = ptp.tile([P, n], F8, tag=f"pt{c}", name=f"pt{c}")
                pick.act_reserve(n * 0.8333 + 185).activation(pt, sT[:, q0:], EXP)
                # causal diagonal: zero exp() where q < k.  For c==0 the diag
                # must be masked in place (its matmul carries start=True and
                # must cover the full bank in one piece); for c>0 mask into a
                # separate tile and order the diag piece last so the nondiag
                # matmuls depend only on exp, not on the Pool-side mask.
                if c == 0:
                    pick.pool_reserve(150).affine_select(
                        out=pt[:, 0:P], in_=pt[:, 0:P],
                        compare_op=mybir.AluOpType.is_ge, fill=0.0,
                        base=0, pattern=[[1, P]], channel_multiplier=-1)
                    ptd = None
                    plist = pieces(c)
                else:
                    ptd = ptp.tile([P, P], F8, tag=f"ptd{c}", name=f"ptd{c}")
                    pick.pool_reserve(150).affine_select(
                        out=ptd, in_=pt[:, 0:P],
                        compare_op=mybir.AluOpType.is_ge, fill=0.0,
                        base=0, pattern=[[1, P]], channel_multiplier=-1)
                    plist = pieces_diag_last(c)
                for (r, lo, hi) in plist:
                    src_ = (ptd if (ptd is not None and lo == q0)
                            else pt[:, lo - q0:hi - q0])
                    nc.tensor.matmul(ytB[r][:, lo - BW * r:hi - BW * r],
                                     v_lhs(c, 0), src_,
                                     start=(c == 0), stop=(c == last_c(r)))
                    nc.tensor.matmul(denB[r][:, lo - BW * r:hi - BW * r],
                                     ones, src_,
                                     start=(c == 0), stop=(c == last_c(r)))
                pts.append((pt, ptd))
            rec = recp.tile([P, T], F16, tag="rec")
            with nc.allow_low_precision(reason="1/den fits fp16; den in [1, 8e3]"):
                for r in range(NR):
                    pick.dve_reserve(BW * 1.0417 + 125).reciprocal(
                        rec[:, BW * r:BW * (r + 1)], denB[r])
            if len(blks) == 2:
                yc0 = ycp.tile([P, T], F16, tag="yc")
                for r in range(NR):
                    pick.psum_drain(yc0[:, BW * r:BW * (r + 1)], ytB[r], BW)
                yt2B = [ypsum.tile([P, BW], F32, tag=f"yt{r}", name=f"yt{r}b")
                        for r in range(NR)]
                for c in range(TK):
                    q0 = P * c
                    plist = pieces(c) if c == 0 else pieces_diag_last(c)
                    for (r, lo, hi) in plist:
                        src_ = (pts[c][1] if (pts[c][1] is not None and lo == q0)
                                else pts[c][0][:, lo - q0:hi - q0])
                        nc.tensor.matmul(yt2B[r][:, lo - BW * r:hi - BW * r],
                                         v_lhs(c, 1), src_,
                                         start=(c == 0), stop=(c == last_c(r)))
                yc1 = ycp.tile([P, T], F16, tag="yc")
                for r in range(NR):
                    pick.psum_drain(yc1[:, BW * r:BW * (r + 1)], yt2B[r], BW)
                for bi, yc in ((0, yc0), (1, yc1)):
                    if store:
                        pick.tt(outacc[:, blks[bi], :], yc, rec, mult, T)
                    else:
                        tmp = scr.tile([P, T], F16, tag="btmp")
                        pick.tt(tmp, yc, rec, mult, T)
                        pick.tt(outacc[:, blks[bi], :], outacc[:, blks[bi], :],
                                tmp, add, T)
                    if do_flush:
                        flush(blks[bi])
            else:
                if store:
                    for r in range(NR):
                        sl = slice(BW * r, BW * (r + 1))
                        pick.tt_psum(outacc[:, blks[0], sl], ytB[r], rec[:, sl], mult, BW)
                else:
                    tmp = scr.tile([P, T], F16, tag="btmp")
                    for r in range(NR):
                        sl = slice(BW * r, BW * (r + 1))
                        pick.tt_psum(tmp[:, sl], ytB[r], rec[:, sl], mult, BW)
                    for r in range(NR):
                        sl = slice(BW * r, BW * (r + 1))
                        pick.tt(outacc[:, blks[0], sl], outacc[:, blks[0], sl],
                                tmp[:, sl], add, BW)
                        if do_flush:  # flush each half as soon as it's ready
                            b = blks[0]
                            nc.sync.dma_start(out=outT[P * b:P * (b + 1), sl],
                                              in_=outacc[:, b, sl])
                    return
                if do_flush:
                    flush(blks[0])

        # ---- per-head builders --------------------------------------
        def do_A(i, store, do_flush, warm=False):
            qeng = nc.scalar if warm else nc.sync
            r1 = slice(P * i, P * (i + 1))
            r2 = slice(256 * i, 256 * (i + 1))
            q1 = get_shared("sq1", D["qT1"][r1, :], i,
                            lambda o, a: qeng.dma_start(out=o, in_=a))
            q1s = rawp.tile([P, T], F16, tag="q1s")
            qeng.dma_start(out=q1s, in_=D["qT1s64"][r1, :])
            q2 = rawp.tile([P, 2, T], F16, tag="q2")
            qeng.dma_start(out=q2, in_=D["qT2"][r2, :].rearrange("(c p) t -> p c t", p=P))
            qmix = mixp.tile([P, 2, T], F16, tag="qmix")
            mix_A(qmix, q1, q1s, q2, tabs["tA1c"], tabs["tA1s"], tabs["tA2c"], tabs["tA2s"])

            k1 = get_shared("sk1", D["kTa1"][r1, :], i)
            k1s = get_shared("sk1s", D["kTa1s64"][r1, :], i)
            k2 = rawp.tile([P, 2, T], F16, tag="k2")
            nc.sync.dma_start(out=k2, in_=D["kTa2"][r2, :].rearrange("(c p) t -> p c t", p=P))
            kmix = mixp.tile([P, 2, T], F16, tag="kmix")
            mix_A(kmix, k1, k1s, k2, tabs["tA1c"], tabs["tA1s"], tabs["tA2c"], tabs["tA2s"])

            v1 = rawp.tile([P, TK, P], F16, tag="v1")
            nc.sync.dma_start(out=v1, in_=D["va1p"][r1, :].rearrange("p (c d) -> p c d", d=P))
            vmix = rawp.tile([P, TK, 2 * P], F16, tag="v2")
            nc.sync.dma_start(out=vmix, in_=D["va2p"][r1, :].rearrange("p (c d) -> p c d", d=2 * P))
            # va2p arrives host-prescaled by w1; fold w0*v1 in-place
            pick.stt(vmix[:, :, 0:P], v1, wv[:, 0:1], vmix[:, :, 0:P],
                     mult, add, T)

            attn([qmix[:, 0, :], qmix[:, 1, :]],
                 [kmix[:, 0, :], kmix[:, 1, :]],
                 lambda c, vc: vmix[:, c, P * vc:P * (vc + 1)],
                 (2 * i, 2 * i + 1), store=store, do_flush=do_flush)

        tp_hold = {}

        def do_B(j, store, do_flush, warm=False):
            rj = slice(P * j, P * (j + 1))
            # warm mode: round-robin the loads over four engine DMA queues so
            # the first head's operands stream in parallel instead of
            # serializing on SP (the model charges transfers per-queue).
            wq = {"k2": nc.sync, "k2s": nc.gpsimd, "k1p": nc.scalar,
                  "k1ps": nc.sync, "q1p": nc.sync, "q1ps": nc.sync,
                  "q2p": nc.scalar, "q2ps": nc.sync, "vmix": nc.sync,
                  "v1": nc.sync}
            wcost = {id(nc.scalar): "act", id(nc.gpsimd): "pool"}

            def dma(out_, in_, key=None):
                eng = wq.get(key, nc.sync) if warm else nc.sync
                eng.dma_start(out=out_, in_=in_)
                if warm and id(eng) in wcost:  # DMA occupies that queue
                    pick.load[wcost[id(eng)]] += out_.free_size() * 2 * 0.3855

            k2 = get_shared("sk1", D["kTa1"][rj, :], j,
                            lambda o, a: dma(o, a, "k2"))
            k2s = get_shared("sk1s", D["kTa1s64"][rj, :], j,
                             lambda o, a: dma(o, a, "k2s"))
            u0 = j // 2
            if j % 2 == 0:
                k1p = rawp.tile([P, T], F16, tag="bk1p")
                dma(k1p, D["kTb1"][P * u0:P * (u0 + 1), :], "k1p")
                k1ps = rawp.tile([P, T], F16, tag="bk1ps")
                dma(k1ps, D["kTb1s32"][P * u0:P * (u0 + 1), :], "k1ps")
            # q pair for heads (2j, 2j+1)
            q1p = get_shared("sq1", D["qT1"][rj, :], j,
                             lambda o, a: dma(o, a, "q1p"))
            q1ps = rawp.tile([P, T], F16, tag="k1s")
            dma(q1ps, D["qT1s32"][rj, :], "q1ps")
            q2p = rawp.tile([P, 2, T], F16, tag="q2")
            dma(q2p, D["qT2"][256 * j:256 * (j + 1), :]
                .rearrange("(c p) t -> p c t", p=P), "q2p")
            q2ps = rawp.tile([P, 2, T], F16, tag="k2")
            dma(q2ps, D["qT2s64"][256 * j:256 * (j + 1), :]
                .rearrange("(c p) t -> p c t", p=P), "q2ps")
            vmix = rawp.tile([P, TK, P], F16, tag="v1")
            dma(vmix, D["va1pw3"][rj, :].rearrange("p (c d) -> p c d", d=P), "vmix")
            v1 = rawp.tile([P, TK, 64], F16, tag="bv1")
            dma(v1, D["vb1p"][rj, :].rearrange("p (c d) -> p c d", d=64), "v1")

            kmix = mixp.tile([P, T], F16, tag="bkmix")
            u = scr.tile([P, T], F16, tag="u1")
            pick.tt(kmix, k2, tabs["tB2c"][:, 0, :], mult, T)
            pick.tt(u, k2s, tabs["tB2s"][:, 0, :], mult, T)
            pick.tt(kmix, kmix, u, add, T)
            if j % 2 == 0:
                tp = scr.tile([P, T], F16, tag="btp")
                tpb = scr.tile([P, T], F16, tag="btpb")
                pick.tt(tp, k1p, tabs["tB1c"][:, 0, :], mult, T)
                pick.tt(tpb, k1ps, tabs["tB1s"][:, 0, :], mult, T)
                pick.tt(tp, tp, tpb, add, T)
                tp_hold[0] = tp
                pick.tt(kmix[0:64, :], kmix[0:64, :], tp[0:64, :], add, T)
            else:
                tc2 = scr.tile([P, T], F16, tag="btc")
                pick.copy(tc2[0:64, :], tp_hold[0][64:128, :], T)
                pick.tt(kmix[0:64, :], kmix[0:64, :], tc2[0:64, :], add, T)

            qp = mixp.tile([P, 2, T], F16, tag="bqp")
            uq = scr.tile([P, 2, T], F16, tag="u2")
            t1 = scr.tile([P, T], F16, tag="u1")
            t1b = scr.tile([P, T], F16, tag="u1b")
            pick.tt(t1, q1p, tabs["tB1c"][:, 0, :], mult, T)
            pick.tt(t1b, q1ps, tabs["tB1s"][:, 0, :], mult, T)
            pick.tt(t1, t1, t1b, add, T)
            for hh in range(2):  # head 0's column completes first
                pick.tt(qp[:, hh, :], q2p[:, hh, :], tabs["tB2c"][:, 0, :],
                        mult, T)
                pick.tt(uq[:, hh, :], q2ps[:, hh, :], tabs["tB2s"][:, 0, :],
                        mult, T)
                pick.tt(qp[:, hh, :], qp[:, hh, :], uq[:, hh, :], add, T)
                if hh == 0:
                    pick.tt(qp[0:64, 0, :], qp[0:64, 0, :], t1[0:64, :], add, T)
                else:
                    tcq = scr.tile([P, T], F16, tag="btc")
                    pick.copy(tcq[0:64, :], t1[64:128, :], T)
                    pick.tt(qp[0:64, 1, :], qp[0:64, 1, :], tcq[0:64, :], add, T)

            # va1pw3 is host-prescaled by w3, vb1p by w2: one in-place add
            pick.tt(vmix[:, :, 0:64], vmix[:, :, 0:64], v1, add, T // 2)

            for hh in range(2):
                attn([qp[:, hh, :]], [kmix],
                     lambda c, vc: vmix[:, c, :],
                     (2 * j + hh,), store=store, do_flush=do_flush)

        # ---- schedule: B pair 0 first (short mix chain covers PE warmup),
        # then A heads (A0 adds onto B0's blocks), then remaining B pairs.
        load_tab("tB2c", nc.gpsimd)
        load_tab("tB2s", nc.scalar)
        load_tab("tB1c", nc.gpsimd)
        load_tab("tB1s", nc.gpsimd)
        pick.load["pool"] += 3 * 790
        pick.load["act"] += 790
        do_B(0, store=True, do_flush=False, warm=True)
        for nm in ("tA1c", "tA1s", "tA2c", "tA2s"):
            load_tab(nm)
        do_A(0, store=False, do_flush=True, warm=True)
        for i in range(1, cfg.NA):
            do_A(i, store=True, do_flush=False)
        for j in range(1, cfg.NKVB):
            do_B(j, store=False, do_flush=True)

    nc.compile()
    return nc


# ---------------------------------------------------------------------------
# Host side
# ---------------------------------------------------------------------------

def _rope_tab(pos, d, f):
    """Transposed rope tables [d, T]: (f*cos, f*sin with rot sign folded)."""
    inv = 1.0 / (10000.0 ** (np.arange(0, d, 2, dtype=np.float32) / d))
    ang = inv[:, None] * pos[None, :].astype(np.float32)      # [d/2, T]
    ang = np.concatenate([ang, ang], 0)                        # [d, T]
    c = (f * np.cos(ang)).astype(np.float32)
    s = (f * np.sin(ang)).astype(np.float32)
    s[: d // 2] *= -1.0
    return c, s


def _sig(a, half):
    """Row permutation: swap halves of size `half` in each 2*half group."""
    out = a.reshape(-1, 2, half, a.shape[-1])[:, ::-1]
    return out.reshape(a.shape)


def _vperm(vslc, dh):
    """[T, nh*dh] -> [nh, P, T//P, dh] contiguous per-partition rows."""
    T = vslc.shape[0]
    nh = vslc.shape[1] // dh
    # [c, p, head, d] -> [head, p, c, d]
    return vslc.reshape(T // P, P, nh, dh).transpose(2, 1, 0, 3)


def make_core_inputs(q, k, v, pos, weights, s, cfg: KCfg = FULL):
    """q,k,v: [T, 2048] fp32 for one batch; returns the per-core input dict."""
    f16 = lambda a: np.ascontiguousarray(a, dtype=np.float16)
    qT1 = q[:, 512 * s:512 * s + 512].T
    qT2 = q[:, 1024 * s:1024 * s + 1024].T
    kTa1 = k[:, 512 * s:512 * s + 512].T
    kTa2 = k[:, 1024 * s:1024 * s + 1024].T
    kTb1 = k[:, 256 * s:256 * s + 256].T
    arrs = {
        "qT1": f16(qT1), "qT1s64": f16(_sig(qT1, 64)), "qT1s32": f16(_sig(qT1, 32)),
        "qT2": f16(qT2), "qT2s64": f16(_sig(qT2, 64)),
        "kTa1": f16(kTa1), "kTa1s64": f16(_sig(kTa1, 64)),
        "kTa2": f16(kTa2),
        "kTb1": f16(kTb1), "kTb1s32": f16(_sig(kTb1, 32)),
        "va1p": f16(_vperm(v[:, 512 * s:512 * s + 512], 128).reshape(4 * P, -1)),
        "va1pw3": f16(float(weights[3])
                      * _vperm(v[:, 512 * s:512 * s + 512], 128).reshape(4 * P, -1)),
        "va2p": f16(float(weights[1])
                    * _vperm(v[:, 1024 * s:1024 * s + 1024], 256).reshape(4 * P, -1)),
        "vb1p": f16(float(weights[2])
                    * _vperm(v[:, 256 * s:256 * s + 256], 64).reshape(4 * P, -1)),
    }
    fA = math.sqrt(1.0 / 16.0)
    fB = math.sqrt(1.0 / math.sqrt(128.0))
    c1, s1 = _rope_tab(pos, 128, fA * float(weights[0]))
    c2, s2 = _rope_tab(pos, 256, fA * float(weights[1]))
    cb1h, sb1h = _rope_tab(pos, 64, fB * float(weights[2]))
    cb2, sb2 = _rope_tab(pos, 128, fB * float(weights[3]))
    arrs.update({
        "tA1c": f16(c1), "tA1s": f16(s1),
        # tA2s half-swapped: row block 0 holds the sin factors for x2[:,0,:]
        # (which contribute to out dim-chunk 1), block 1 those for x2[:,1,:].
        "tA2c": f16(c2), "tA2s": f16(np.vstack([s2[128:], s2[:128]])),
        "tB1c": f16(np.vstack([cb1h, cb1h])), "tB1s": f16(np.vstack([sb1h, sb1h])),
        "tB2c": f16(cb2), "tB2s": f16(sb2),
        "wvec": np.tile(np.asarray(weights, np.float32)[None, :], (P, 1)),
    })
    return arrs


_PROGRAM_CACHE = {}
TRACE = False
LAST_RESULT = None


def kernel(q_m, k_m, v_m, weights, attention_mask, position_ids):
    global LAST_RESULT
    from concourse.bass_utils import run_bass_kernel_spmd

    cfg = FULL
    q_m = np.asarray(q_m, np.float32)
    k_m = np.asarray(k_m, np.float32)
    v_m = np.asarray(v_m, np.float32)
    weights = np.asarray(weights, np.float32)
    attention_mask = np.asarray(attention_mask, np.float32)
    position_ids = np.asarray(position_ids)
    B, T, H = q_m.shape

    # the device program hardcodes the causal structure; verify it holds
    causal = np.where(np.tril(np.ones((T, T), bool)), 0.0, NEG).astype(np.float32)
    for b in range(B):
        assert np.array_equal(attention_mask[b, 0], causal), "non-causal mask"

    if "nc" not in _PROGRAM_CACHE:
        _PROGRAM_CACHE["nc"] = build_program(cfg)
    nc = _PROGRAM_CACHE["nc"]

    in_maps = []
    for b in range(B):
        for s in range(2):
            in_maps.append(make_core_inputs(
                q_m[b], k_m[b], v_m[b], position_ids[b], weights, s, cfg))
    res = run_bass_kernel_spmd(nc, in_maps, list(range(8)), trace=TRACE)
    LAST_RESULT = res
    out = np.zeros((B, T, H), np.float32)
    for b in range(B):
        for s in range(2):
            out[b, :, 1024 * s:1024 * s + 1024] = \
                res.results[2 * b + s]["outT"].astype(np.float32).T
    return out
